# revision 12
# baseline (speedup 1.0000x reference)
"""Trainium2 Bass kernel for nn_ConvSurface: barycentric surface sampling +
3->64 linear map + ReLU + max over 24 samples.

Sharding: face dimension across 8 cores (alpha/beta/gamma shard too).
Per core: F=2048 faces x M=8 meshes (fm = m*2048 + f, mesh-major).

Device pipeline per core (bf16 compute, f32 PSUM):
  1. DMA in: corn [128,3456] f32 (layout [i,d,f,n] per partition),
     cent [128,384] f32 ([f,d]), coefa/b/g [128,3072] bf16 ([f,s]),
     wblk [6,128] bf16 (block-diag W^T x2)
  2. DVE: cd = corn - cent (3 subs, one per d) -> bf16 [i,d,f,n]
  3. DVE: dirs[d,f,s] = sum_i coef_i[f,s] * cd[i,d,f]  (per-d mults+adds;
     the t-broadcast of cd rides as a 0-step AP dim). alpha+beta+gamma=1
     folds the -center into cd.
  4. SBUF->SBUF DMA: repack dirs into PE rhs layout
     [rows 32k+3eo+d, fm_local*24] in two half-tiles (big coalesced DMAs)
  5. PE: fea = dirs . W via 4x row-tiled (32x128) bf16 matmuls, N=384
  6. Drain: mix of (A) DVE reduce_max straight from PSUM and
     (B) ACT relu-pass to SBUF bf16 + DVE pairwise-max tree
  7. DMA out bf16 [128=(eo,k), 8192=(rg,g,floc)]; host un-shuffles.
"""

import json
import sys
import types

import numpy as np

sys.path.insert(0, "/opt/trn_rl_repo")

NUM_MESHES = 8
NUM_FACES = 16384
NUM_KERNEL = 64
N_CORES = 8

F = NUM_FACES // N_CORES          # 2048 faces per core
FM = NUM_MESHES * F               # 16384 face-mesh pairs per core
FL = FM // 128                    # 128 fm-items per partition
S = 24

N_MM = 384                        # 16 faces x 24 samples per matmul
FACES_PER_MM = 16
RHS_FREE = 8 * FL * S             # rhs half-tile free size 24576
MM_PER_HALF_RG = (8 * 128 * S) // N_MM  # 64

# Per-PAIR drain routes (a pair = 2 batches = 4 PSUM tiles = 512 outputs):
# 'R' = DVE tensor_reduce direct from PSUM -> osb (no ACT),
# 'B' = 4x ACT relu -> s-major fsb [s(24), g(256)] + 5-op in-place DVE
#       max tree (24->12->6->3->1, all contiguous bf16 2x ops).
# 8 per (H,h) quarter, 32 total; tuned so DVE/ACT busy times balance.
ROUTE_PATTERN = ['B', 'B', 'R', 'B',
                 'B', 'B', 'B', 'B']


# --------------------------------------------------------------------------
# Harness patches (wait-split for walrus 1-wait limit; NTFF profiling shim)
# --------------------------------------------------------------------------

def _split_waits(bir: dict) -> dict:
    """walrus codegen accepts at most 1 sync wait per instruction (2 for
    EventSemaphore); Tile sometimes emits more. Move the excess onto NoOp
    carriers inserted just before the instruction on the same engine."""
    n = [0]
    for fn in bir.get("functions", []):
        for bb in fn.get("blocks", []):
            out = []
            for inst in bb.get("instructions", []):
                si = inst.get("sync_info") or {}
                waits = si.get("on_wait") or []
                cap = 2 if inst.get("opcode") == "EventSemaphore" else 1
                if len(waits) > cap:
                    for w in waits[cap:]:
                        n[0] += 1
                        out.append({
                            "name": f"wsplit-{n[0]}",
                            "opcode": "NoOp",
                            "engine": inst.get("engine"),
                            "ins": [], "outs": [],
                            "debug": inst.get("debug"),
                            "sync_info": {"on_update": [], "on_wait": [w]},
                        })
                    si["on_wait"] = waits[:cap]
                    inst["sync_info"] = si
                out.append(inst)
            bb["instructions"] = out
    return bir


def _install_patches():
    import concourse.bass_utils as bu
    import concourse.bass2jax as b2j
    if not getattr(bu, "_wsplit_installed", False):
        orig = bu.compile_bir_kernel

        def wrapper(bir_str, *a, **kw):
            if isinstance(bir_str, (bytes, bytearray)):
                bir_str = json.dumps(_split_waits(json.loads(bir_str))).encode()
            elif isinstance(bir_str, str):
                bir_str = json.dumps(_split_waits(json.loads(bir_str)))
            return orig(bir_str, *a, **kw)

        bu.compile_bir_kernel = wrapper
        b2j.compile_bir_kernel = wrapper
        bu._wsplit_installed = True

    if "antenv.axon_hooks" not in sys.modules:
        mod = types.ModuleType("antenv.axon_hooks")
        _hook = [None]
        mod.set_axon_ntff_profile_hook = lambda h: _hook.__setitem__(0, h)
        mod.get_axon_ntff_profile_hook = lambda: _hook[0]
        sys.modules["antenv.axon_hooks"] = mod
        try:
            import antenv
            antenv.axon_hooks = mod
            from trn_agent_boot.trn_boot import _ntff_profile_via_ctypes
            mod.set_axon_ntff_profile_hook(
                _ntff_profile_via_ctypes("/opt/axon/libaxon_pjrt.so"))
        except Exception:
            pass


# --------------------------------------------------------------------------
# Device kernel
# --------------------------------------------------------------------------

def _merge_ap(ap_obj):
    """Merge adjacent free dims (outer.step == inner.step*inner.count), drop
    count-1 dims -> fit the 3-free-dim ISA mem-pattern limit."""
    import concourse.bass as bass
    pairs = [list(p) for p in ap_obj.ap]
    part, rest = pairs[0], pairs[1:]
    merged = []
    for s, c in rest:
        if c == 1:
            continue
        if merged and merged[-1][0] == s * c:
            merged[-1] = [s, merged[-1][1] * c]
        else:
            merged.append([s, c])
    if not merged:
        merged = [[1, 1]]
    return bass.AP(ap_obj.tensor, ap_obj.offset, [part] + merged)


def _build_nc():
    import concourse.bass as bass
    import concourse.tile as tile
    from concourse import mybir

    f32 = mybir.dt.float32
    bf16 = mybir.dt.bfloat16
    nc = bass.Bass()

    corn_d = nc.declare_dram_parameter("corn", [128, FL * 27], bf16, isOutput=False)
    cent_d = nc.declare_dram_parameter("cent", [128, FL * 3], bf16, isOutput=False)
    coef_d = [nc.declare_dram_parameter(f"coef{i}", [128, FL * S], bf16,
                                        isOutput=False) for i in range(3)]
    wblk_d = nc.declare_dram_parameter("wblk", [6, 128], bf16, isOutput=False)
    out_d = nc.declare_dram_parameter("out", [128, FM // 2], bf16, isOutput=True)

    AX = mybir.AluOpType
    FH = FL // 2                   # f-local per H-half (64)
    RHS_H = 8 * FH * S             # rhs free per (H, h) tile = 12288
    MM_PER = RHS_H // N_MM         # 32 matmuls per (H, h, rg)

    with tile.TileContext(nc) as tc:
        with (
            tc.tile_pool(name="inputs", bufs=1) as inp_pool,
            tc.tile_pool(name="w", bufs=1) as w_pool,
            tc.tile_pool(name="dirs", bufs=2) as dirs_pool,
            tc.tile_pool(name="tmp", bufs=1) as tmp_pool,
            tc.tile_pool(name="rhs", bufs=3) as rhs_pool,
            tc.tile_pool(name="fsb", bufs=3) as fsb_pool,
            tc.tile_pool(name="dtree", bufs=2) as dtree_pool,
            tc.tile_pool(name="osb", bufs=2) as osb_pool,
            tc.tile_pool(name="psum", bufs=2, space="PSUM") as psum_pool,
        ):
            # ---- loads -------------------------------------------------
            corn = inp_pool.tile([128, FL * 27], bf16)    # [i, d, f, n]
            nc.sync.dma_start(corn[:], corn_d[:])
            cent = inp_pool.tile([128, FL * 3], bf16)     # [f, d]
            nc.sync.dma_start(cent[:], cent_d[:])
            coef = []
            for i in range(3):
                t = inp_pool.tile([128, FL * S], bf16, tag=f"coef{i}")  # [f, s]
                nc.sync.dma_start(t[:], coef_d[i][:])
                coef.append(t)
            wt = w_pool.tile([128, 128], bf16)
            for rg in range(4):
                nc.sync.dma_start(wt[32 * rg:32 * rg + 6, :], wblk_d[:, :])

            # ---- cd = corn - cent (in place, bf16, layout [i, d, f, n]) -
            cr5 = corn[:].rearrange("p (i d f n) -> p i d f n", i=3, d=3, f=FL, n=3)
            ce3 = cent[:].rearrange("p (f d) -> p f d", f=FL, d=3)
            for d in range(3):
                ce = ce3[:, :, d].unsqueeze(1).unsqueeze(3)
                ce = ce.broadcast_to((128, 3, FL, 3))
                nc.vector.tensor_tensor(
                    _merge_ap(cr5[:, :, d, :, :]), _merge_ap(cr5[:, :, d, :, :]),
                    _merge_ap(ce), op=AX.subtract)
            # gamma-elimination: cda = cd0 - cd2, cdb = cd1 - cd2 (in place
            # over the i=0,1 slots; i=2 keeps cd2). dirs = cd2 + a*cda + b*cdb
            # (alpha+beta+gamma = 1).
            for i in range(2):
                nc.vector.tensor_tensor(
                    _merge_ap(cr5[:, i, :, :, :]), _merge_ap(cr5[:, i, :, :, :]),
                    _merge_ap(cr5[:, 2, :, :, :]), op=AX.subtract)

            batch_idx = 0
            for H in range(2):
                # ---- dirs[d][f(FH), s] for this f-half ------------------
                fsl = slice(H * FH, (H + 1) * FH)
                dirs = []
                for d in range(3):
                    dt_ = dirs_pool.tile([128, FH * S], bf16, tag=f"dirs{d}")
                    dirs.append(dt_)
                for d in range(3):

                    def cd_ap(i):
                        a = cr5[:, i, d, fsl, :]           # p f n
                        a = a.unsqueeze(2).broadcast_to((128, FH, 8, 3))
                        return _merge_ap(a)

                    csl = slice(H * FH * S, (H + 1) * FH * S)
                    t1 = tmp_pool.tile([128, FH * S], bf16, tag="t1")
                    nc.vector.tensor_mul(t1[:], coef[0][:, csl], cd_ap(0))
                    t2 = tmp_pool.tile([128, FH * S], bf16, tag="t2")
                    nc.vector.tensor_mul(t2[:], coef[1][:, csl], cd_ap(1))
                    nc.vector.tensor_add(t1[:], t1[:], t2[:])
                    nc.vector.tensor_add(dirs[d][:], t1[:], cd_ap(2))

                for h in range(2):
                    rhs = rhs_pool.tile([128, RHS_H], bf16)
                    for eo in range(2):
                        for d in range(3):
                            for k in range(4):
                                src = dirs[d][32 * k + 16 * eo + 8 * h:
                                              32 * k + 16 * eo + 8 * h + 8, :]
                                dst = rhs[32 * k + 3 * eo + d:
                                          32 * k + 3 * eo + d + 1, :]
                                dst = dst.rearrange("p (j c) -> p j c", j=8, c=FH * S)
                                nc.sync.dma_start(dst, src)

                    osb = osb_pool.tile([128, 2048], bf16)
                    for pr in range(8):     # pairs: 4 quads x 4 MMs each
                        route = ROUTE_PATTERN[batch_idx % 8]
                        batch_idx += 1
                        fsb = (fsb_pool.tile([128, 6144], bf16, name="fsb")
                               if route == "B" else None)
                        for j in range(4):
                            q = pr * 4 + j
                            ps = psum_pool.tile([128, 2048], f32)
                            for u in range(4):   # u = row group (rotates!)
                                nc.tensor.matmul(
                                    ps[:, u * 512:u * 512 + N_MM],
                                    wt[32 * u:32 * u + 6, :],
                                    rhs[32 * u:32 * u + 6,
                                        q * N_MM:(q + 1) * N_MM],
                                    start=True, stop=True,
                                    tile_position=(32 * u, 0))
                            pa = bass.AP(
                                ps[:].tensor, ps[:].offset,
                                [list(ps[:].ap[0]),
                                 [512, 4], [S, FACES_PER_MM], [1, S]])
                            if route == "R":
                                osl = osb[:, q * 64:(q + 1) * 64]
                                nc.vector.tensor_reduce(
                                    osl.rearrange("p (u f) -> p u f", u=4),
                                    pa, axis=mybir.AxisListType.X, op=AX.max)
                            else:
                                # fsb written s-major: col = s*256 + j*64
                                #  + u*16 + f  (so tree levels are contiguous)
                                ob = bass.AP(
                                    fsb[:].tensor, fsb[:].offset + j * 64,
                                    [list(fsb[:].ap[0]),
                                     [16, 4], [1, FACES_PER_MM], [256, S]])
                                nc.scalar.activation(
                                    ob, pa, mybir.ActivationFunctionType.Relu)
                        if route == "B":
                            # in-place DVE max tree: s 24->12->6->3->1, all
                            # contiguous bf16 (2x mode)
                            osl = osb[:, pr * 256:(pr + 1) * 256]
                            nc.vector.tensor_tensor(
                                fsb[:, 0:3072], fsb[:, 0:3072],
                                fsb[:, 3072:6144], op=AX.max)
                            nc.vector.tensor_tensor(
                                fsb[:, 0:1536], fsb[:, 0:1536],
                                fsb[:, 1536:3072], op=AX.max)
                            nc.vector.tensor_tensor(
                                fsb[:, 0:768], fsb[:, 0:768],
                                fsb[:, 768:1536], op=AX.max)
                            nc.vector.tensor_tensor(
                                osl, fsb[:, 0:256], fsb[:, 256:512], op=AX.max)
                            nc.vector.tensor_tensor(
                                osl, osl, fsb[:, 512:768], op=AX.max)
                    # relu for 'R' pairs (idempotent on B outputs)
                    nc.vector.tensor_scalar_max(osb[:], osb[:], 0.0)
                    nc.sync.dma_start(
                        out_d[:, (h * 2 + H) * 2048:(h * 2 + H + 1) * 2048],
                        osb[:])
    return nc


_CACHE = {}


def _get_nc():
    if "nc" not in _CACHE:
        _install_patches()
        _CACHE["nc"] = _build_nc()
    return _CACHE["nc"]


# --------------------------------------------------------------------------
# Host wrapper
# --------------------------------------------------------------------------

def _prep_core_inputs(centers, neighbor_corners, alpha, beta, gamma, W, c):
    import ml_dtypes
    bf = ml_dtypes.bfloat16
    fsl = slice(c * F, (c + 1) * F)
    cent = np.ascontiguousarray(
        centers[:, fsl].reshape(128, FL, 3),
        dtype=np.float32).reshape(128, FL * 3).astype(bf)
    # corn per-partition rows [f, n, i, d] -> [i, d, f, n]
    cr = neighbor_corners[:, fsl].reshape(128, FL, 3, 3, 3)
    corn = np.ascontiguousarray(cr.transpose(0, 3, 4, 1, 2), dtype=np.float32)
    corn = corn.reshape(128, FL * 27).astype(bf)
    cf = []
    for arr in (alpha, beta, gamma):
        a = np.tile(arr[fsl].reshape(1, F, S), (NUM_MESHES, 1, 1))
        cf.append(np.ascontiguousarray(
            a.reshape(128, FL * S), dtype=np.float32).astype(bf))
    wblk = np.zeros((6, 128), dtype=np.float32)
    wblk[0:3, 0:64] = W.T
    wblk[3:6, 64:128] = W.T
    return {"corn": corn, "cent": cent,
            "coef0": cf[0], "coef1": cf[1], "coef2": cf[2],
            "wblk": wblk.astype(bf)}


def _unshuffle_core_out(raw):
    # raw [128=(eo,k), 8192]; col = (h*2+H)*2048 + q*64 + u*16 + f
    # stream e = 2u+eo; fm_local = h*1024 + j*128 + H*64 + fl, (q*16+f)=j*64+fl
    r = np.asarray(raw, dtype=np.float32).reshape(2, 64, 2, 2, 32, 4, 16)
    r = r.transpose(5, 0, 2, 4, 6, 3, 1)        # u eo h q f H k
    r = r.reshape(4, 2, 2, 8, 64, 2, 64)        # u eo h j fl H k
    r = r.transpose(0, 1, 2, 3, 5, 4, 6)        # u eo h j H fl k
    return np.ascontiguousarray(r).reshape(NUM_MESHES, F, NUM_KERNEL)


def run(inputs, trace=False):
    from concourse.bass_utils import run_bass_kernel_spmd
    nc = _get_nc()
    centers = np.asarray(inputs["centers"], dtype=np.float32)
    corners = np.asarray(inputs["neighbor_corners"], dtype=np.float32)
    alpha = np.asarray(inputs["alpha"], dtype=np.float32)
    beta = np.asarray(inputs["beta"], dtype=np.float32)
    gamma = np.asarray(inputs["gamma"], dtype=np.float32)
    W = np.asarray(inputs["W"], dtype=np.float32)

    in_maps = [
        _prep_core_inputs(centers, corners, alpha, beta, gamma, W, c)
        for c in range(N_CORES)
    ]
    res = run_bass_kernel_spmd(
        nc, in_maps, core_ids=list(range(N_CORES)), trace=trace)
    out = np.empty((NUM_MESHES, NUM_FACES, NUM_KERNEL), dtype=np.float32)
    for c in range(N_CORES):
        out[:, c * F:(c + 1) * F, :] = _unshuffle_core_out(res.results[c]["out"])
    return out, res


def kernel(**inputs) -> np.ndarray:
    out, _ = run(inputs, trace=False)
    return out



# revision 16
# speedup vs baseline: 2.9425x; 2.9425x over previous
"""Trainium2 Bass kernel for nn_ConvSurface: barycentric surface sampling +
3->64 linear map + ReLU + max over 24 samples.

Sharding: face dimension across 8 cores (alpha/beta/gamma shard too).
Per core: F=2048 faces x M=8 meshes (fm = m*2048 + f, mesh-major).

Device pipeline per core (bf16 compute, f32 PSUM):
  1. DMA in: corn [128,3456] f32 (layout [i,d,f,n] per partition),
     cent [128,384] f32 ([f,d]), coefa/b/g [128,3072] bf16 ([f,s]),
     wblk [6,128] bf16 (block-diag W^T x2)
  2. DVE: cd = corn - cent (3 subs, one per d) -> bf16 [i,d,f,n]
  3. DVE: dirs[d,f,s] = sum_i coef_i[f,s] * cd[i,d,f]  (per-d mults+adds;
     the t-broadcast of cd rides as a 0-step AP dim). alpha+beta+gamma=1
     folds the -center into cd.
  4. SBUF->SBUF DMA: repack dirs into PE rhs layout
     [rows 32k+3eo+d, fm_local*24] in two half-tiles (big coalesced DMAs)
  5. PE: fea = dirs . W via 4x row-tiled (32x128) bf16 matmuls, N=384
  6. Drain: mix of (A) DVE reduce_max straight from PSUM and
     (B) ACT relu-pass to SBUF bf16 + DVE pairwise-max tree
  7. DMA out bf16 [128=(eo,k), 8192=(rg,g,floc)]; host un-shuffles.
"""

import json
import sys
import types

import numpy as np

sys.path.insert(0, "/opt/trn_rl_repo")

NUM_MESHES = 8
NUM_FACES = 16384
NUM_KERNEL = 64
N_CORES = 8

F = NUM_FACES // N_CORES          # 2048 faces per core
FM = NUM_MESHES * F               # 16384 face-mesh pairs per core
FL = FM // 128                    # 128 fm-items per partition
S = 24

N_MM = 384                        # 16 faces x 24 samples per matmul
FACES_PER_MM = 16
RHS_FREE = 8 * FL * S             # rhs half-tile free size 24576
MM_PER_HALF_RG = (8 * 128 * S) // N_MM  # 64

# Per-PAIR drain routes (a pair = 2 batches = 4 PSUM tiles = 512 outputs):
# 'R' = DVE tensor_reduce direct from PSUM -> osb (no ACT),
# 'B' = 4x ACT relu -> fsb [g(256), s(24)] (contiguous writes) + DVE max
#       tree (pairwise 24->12->6->3 at 2x + reduce-3 at 1x).
# 8 per (H,h) quarter, 32 total; tuned so DVE/ACT busy times balance.
ROUTE_PATTERN = ['B', 'B', 'R', 'B',
                 'B', 'B', 'B', 'B']


# --------------------------------------------------------------------------
# Harness patches (wait-split for walrus 1-wait limit; NTFF profiling shim)
# --------------------------------------------------------------------------

def _split_waits(bir: dict) -> dict:
    """walrus codegen accepts at most 1 sync wait per instruction (2 for
    EventSemaphore); Tile sometimes emits more. Move the excess onto NoOp
    carriers inserted just before the instruction on the same engine."""
    n = [0]
    for fn in bir.get("functions", []):
        for bb in fn.get("blocks", []):
            out = []
            for inst in bb.get("instructions", []):
                si = inst.get("sync_info") or {}
                waits = si.get("on_wait") or []
                cap = 2 if inst.get("opcode") == "EventSemaphore" else 1
                if len(waits) > cap:
                    for w in waits[cap:]:
                        n[0] += 1
                        out.append({
                            "name": f"wsplit-{n[0]}",
                            "opcode": "NoOp",
                            "engine": inst.get("engine"),
                            "ins": [], "outs": [],
                            "debug": inst.get("debug"),
                            "sync_info": {"on_update": [], "on_wait": [w]},
                        })
                    si["on_wait"] = waits[:cap]
                    inst["sync_info"] = si
                out.append(inst)
            bb["instructions"] = out
    return bir


def _install_patches():
    import concourse.bass_utils as bu
    import concourse.bass2jax as b2j
    if not getattr(bu, "_wsplit_installed", False):
        orig = bu.compile_bir_kernel

        def wrapper(bir_str, *a, **kw):
            if isinstance(bir_str, (bytes, bytearray)):
                bir_str = json.dumps(_split_waits(json.loads(bir_str))).encode()
            elif isinstance(bir_str, str):
                bir_str = json.dumps(_split_waits(json.loads(bir_str)))
            return orig(bir_str, *a, **kw)

        bu.compile_bir_kernel = wrapper
        b2j.compile_bir_kernel = wrapper
        bu._wsplit_installed = True

    if "antenv.axon_hooks" not in sys.modules:
        mod = types.ModuleType("antenv.axon_hooks")
        _hook = [None]
        mod.set_axon_ntff_profile_hook = lambda h: _hook.__setitem__(0, h)
        mod.get_axon_ntff_profile_hook = lambda: _hook[0]
        sys.modules["antenv.axon_hooks"] = mod
        try:
            import antenv
            antenv.axon_hooks = mod
            from trn_agent_boot.trn_boot import _ntff_profile_via_ctypes
            mod.set_axon_ntff_profile_hook(
                _ntff_profile_via_ctypes("/opt/axon/libaxon_pjrt.so"))
        except Exception:
            pass


# --------------------------------------------------------------------------
# Device kernel
# --------------------------------------------------------------------------

def _merge_ap(ap_obj):
    """Merge adjacent free dims (outer.step == inner.step*inner.count), drop
    count-1 dims -> fit the 3-free-dim ISA mem-pattern limit."""
    import concourse.bass as bass
    pairs = [list(p) for p in ap_obj.ap]
    part, rest = pairs[0], pairs[1:]
    merged = []
    for s, c in rest:
        if c == 1:
            continue
        if merged and merged[-1][0] == s * c:
            merged[-1] = [s, merged[-1][1] * c]
        else:
            merged.append([s, c])
    if not merged:
        merged = [[1, 1]]
    return bass.AP(ap_obj.tensor, ap_obj.offset, [part] + merged)


def _build_nc():
    import concourse.bass as bass
    import concourse.tile as tile
    from concourse import mybir

    f32 = mybir.dt.float32
    bf16 = mybir.dt.bfloat16
    nc = bass.Bass()

    corn_d = nc.declare_dram_parameter("corn", [128, FL * 27], bf16, isOutput=False)
    cent_d = nc.declare_dram_parameter("cent", [128, FL * 3], bf16, isOutput=False)
    coef_d = [nc.declare_dram_parameter(f"coef{i}", [128, FL * S], bf16,
                                        isOutput=False) for i in range(3)]
    wblk_d = nc.declare_dram_parameter("wblk", [6, 128], bf16, isOutput=False)
    out_d = nc.declare_dram_parameter("out", [128, FM // 2], bf16, isOutput=True)

    AX = mybir.AluOpType
    FH = FL // 2                   # f-local per H-half (64)
    RHS_H = 8 * FH * S             # rhs free per (H, h) tile = 12288
    MM_PER = RHS_H // N_MM         # 32 matmuls per (H, h, rg)

    with tile.TileContext(nc) as tc:
        with (
            tc.tile_pool(name="inputs", bufs=1) as inp_pool,
            tc.tile_pool(name="w", bufs=1) as w_pool,
            tc.tile_pool(name="dirs", bufs=2) as dirs_pool,
            tc.tile_pool(name="tmp", bufs=1) as tmp_pool,
            tc.tile_pool(name="rhs", bufs=3) as rhs_pool,
            tc.tile_pool(name="fsb", bufs=3) as fsb_pool,
            tc.tile_pool(name="dtree", bufs=2) as dtree_pool,
            tc.tile_pool(name="osb", bufs=2) as osb_pool,
            tc.tile_pool(name="psum", bufs=2, space="PSUM") as psum_pool,
        ):
            # ---- loads -------------------------------------------------
            corn = inp_pool.tile([128, FL * 27], bf16)    # [i, d, f, n]
            nc.sync.dma_start(corn[:], corn_d[:])
            cent = inp_pool.tile([128, FL * 3], bf16)     # [f, d]
            nc.sync.dma_start(cent[:], cent_d[:])
            coef = []
            for i in range(3):
                t = inp_pool.tile([128, FL * S], bf16, tag=f"coef{i}")  # [f, s]
                nc.sync.dma_start(t[:], coef_d[i][:])
                coef.append(t)
            wt = w_pool.tile([128, 128], bf16)
            for rg in range(4):
                nc.sync.dma_start(wt[32 * rg:32 * rg + 6, :], wblk_d[:, :])

            # ---- cd = corn - cent (in place, bf16, layout [i, d, f, n]) -
            cr5 = corn[:].rearrange("p (i d f n) -> p i d f n", i=3, d=3, f=FL, n=3)
            ce3 = cent[:].rearrange("p (f d) -> p f d", f=FL, d=3)
            for d in range(3):
                ce = ce3[:, :, d].unsqueeze(1).unsqueeze(3)
                ce = ce.broadcast_to((128, 3, FL, 3))
                nc.vector.tensor_tensor(
                    _merge_ap(cr5[:, :, d, :, :]), _merge_ap(cr5[:, :, d, :, :]),
                    _merge_ap(ce), op=AX.subtract)
            # gamma-elimination: cda = cd0 - cd2, cdb = cd1 - cd2 (in place
            # over the i=0,1 slots; i=2 keeps cd2). dirs = cd2 + a*cda + b*cdb
            # (alpha+beta+gamma = 1).
            for i in range(2):
                nc.vector.tensor_tensor(
                    _merge_ap(cr5[:, i, :, :, :]), _merge_ap(cr5[:, i, :, :, :]),
                    _merge_ap(cr5[:, 2, :, :, :]), op=AX.subtract)

            batch_idx = 0
            for H in range(2):
                # ---- dirs[d][f(FH), s] for this f-half ------------------
                fsl = slice(H * FH, (H + 1) * FH)
                dirs = []
                for d in range(3):
                    dt_ = dirs_pool.tile([128, FH * S], bf16, tag=f"dirs{d}")
                    dirs.append(dt_)
                for d in range(3):

                    def cd_ap(i):
                        a = cr5[:, i, d, fsl, :]           # p f n
                        a = a.unsqueeze(2).broadcast_to((128, FH, 8, 3))
                        return _merge_ap(a)

                    csl = slice(H * FH * S, (H + 1) * FH * S)
                    t1 = tmp_pool.tile([128, FH * S], bf16, tag="t1")
                    nc.vector.tensor_mul(t1[:], coef[0][:, csl], cd_ap(0))
                    t2 = tmp_pool.tile([128, FH * S], bf16, tag="t2")
                    nc.vector.tensor_mul(t2[:], coef[1][:, csl], cd_ap(1))
                    nc.vector.tensor_add(t1[:], t1[:], t2[:])
                    nc.vector.tensor_add(dirs[d][:], t1[:], cd_ap(2))

                for h in range(2):
                    rhs = rhs_pool.tile([128, RHS_H], bf16)
                    for eo in range(2):
                        for d in range(3):
                            for k in range(4):
                                src = dirs[d][32 * k + 16 * eo + 8 * h:
                                              32 * k + 16 * eo + 8 * h + 8, :]
                                dst = rhs[32 * k + 3 * eo + d:
                                          32 * k + 3 * eo + d + 1, :]
                                dst = dst.rearrange("p (j c) -> p j c", j=8, c=FH * S)
                                nc.sync.dma_start(dst, src)

                    osb = osb_pool.tile([128, 2048], bf16)
                    for pr in range(8):     # pairs: 4 quads x 4 MMs each
                        route = ROUTE_PATTERN[batch_idx % 8]
                        batch_idx += 1
                        fsb = (fsb_pool.tile([128, 6144], bf16, name="fsb")
                               if route == "B" else None)
                        for j in range(4):
                            q = pr * 4 + j
                            ps = psum_pool.tile([128, 2048], f32)
                            for u in range(4):   # u = row group (rotates!)
                                nc.tensor.matmul(
                                    ps[:, u * 512:u * 512 + N_MM],
                                    wt[32 * u:32 * u + 6, :],
                                    rhs[32 * u:32 * u + 6,
                                        q * N_MM:(q + 1) * N_MM],
                                    start=True, stop=True,
                                    tile_position=(32 * u, 0))
                            pa = bass.AP(
                                ps[:].tensor, ps[:].offset,
                                [list(ps[:].ap[0]),
                                 [512, 4], [S, FACES_PER_MM], [1, S]])
                            if route == "R":
                                osl = osb[:, q * 64:(q + 1) * 64]
                                nc.vector.tensor_reduce(
                                    osl.rearrange("p (u f) -> p u f", u=4),
                                    pa, axis=mybir.AxisListType.X, op=AX.max)
                            else:
                                nc.scalar.activation(
                                    fsb[:, j * 1536:(j + 1) * 1536].rearrange(
                                        "p (u c) -> p u c", u=4),
                                    bass.AP(ps[:].tensor, ps[:].offset,
                                            [list(ps[:].ap[0]),
                                             [512, 4], [1, N_MM]]),
                                    mybir.ActivationFunctionType.Relu)
                        if route == "B":
                            # DVE max tree over pair: [g=256, s=24]
                            osl = osb[:, pr * 256:(pr + 1) * 256]
                            f3 = fsb[:].rearrange("p (g s) -> p g s", g=256, s=S)
                            tr1 = dtree_pool.tile([128, 3072], bf16, tag="tr1")
                            nc.vector.tensor_tensor(
                                tr1[:], _merge_ap(f3[:, :, 0:12]),
                                _merge_ap(f3[:, :, 12:24]), op=AX.max)
                            t13 = tr1[:].rearrange("p (g s) -> p g s", g=256, s=12)
                            tr2 = dtree_pool.tile([128, 1536], bf16, tag="tr2")
                            nc.vector.tensor_tensor(
                                tr2[:], _merge_ap(t13[:, :, 0:6]),
                                _merge_ap(t13[:, :, 6:12]), op=AX.max)
                            t23 = tr2[:].rearrange("p (g s) -> p g s", g=256, s=6)
                            tr3 = dtree_pool.tile([128, 768], bf16, tag="tr3")
                            nc.vector.tensor_tensor(
                                tr3[:], _merge_ap(t23[:, :, 0:3]),
                                _merge_ap(t23[:, :, 3:6]), op=AX.max)
                            nc.vector.tensor_reduce(
                                osl,
                                tr3[:].rearrange("p (g s) -> p g s", g=256, s=3),
                                axis=mybir.AxisListType.X, op=AX.max)
                    # relu for 'R' pairs (idempotent on B outputs)
                    nc.vector.tensor_scalar_max(osb[:], osb[:], 0.0)
                    nc.sync.dma_start(
                        out_d[:, (h * 2 + H) * 2048:(h * 2 + H + 1) * 2048],
                        osb[:])
    return nc


_CACHE = {}


def _get_nc():
    if "nc" not in _CACHE:
        _install_patches()
        _CACHE["nc"] = _build_nc()
    return _CACHE["nc"]


# --------------------------------------------------------------------------
# Host wrapper
# --------------------------------------------------------------------------

def _prep_core_inputs(centers, neighbor_corners, alpha, beta, gamma, W, c):
    import ml_dtypes
    bf = ml_dtypes.bfloat16
    fsl = slice(c * F, (c + 1) * F)
    cent = np.ascontiguousarray(
        centers[:, fsl].reshape(128, FL, 3),
        dtype=np.float32).reshape(128, FL * 3).astype(bf)
    # corn per-partition rows [f, n, i, d] -> [i, d, f, n]
    cr = neighbor_corners[:, fsl].reshape(128, FL, 3, 3, 3)
    corn = np.ascontiguousarray(cr.transpose(0, 3, 4, 1, 2), dtype=np.float32)
    corn = corn.reshape(128, FL * 27).astype(bf)
    cf = []
    for arr in (alpha, beta, gamma):
        a = np.tile(arr[fsl].reshape(1, F, S), (NUM_MESHES, 1, 1))
        cf.append(np.ascontiguousarray(
            a.reshape(128, FL * S), dtype=np.float32).astype(bf))
    wblk = np.zeros((6, 128), dtype=np.float32)
    wblk[0:3, 0:64] = W.T
    wblk[3:6, 64:128] = W.T
    return {"corn": corn, "cent": cent,
            "coef0": cf[0], "coef1": cf[1], "coef2": cf[2],
            "wblk": wblk.astype(bf)}


def _unshuffle_core_out(raw):
    # raw [128=(eo,k), 8192]; col = (h*2+H)*2048 + q*64 + u*16 + f
    # stream e = 2u+eo; fm_local = h*1024 + j*128 + H*64 + fl, (q*16+f)=j*64+fl
    r = np.asarray(raw, dtype=np.float32).reshape(2, 64, 2, 2, 32, 4, 16)
    r = r.transpose(5, 0, 2, 4, 6, 3, 1)        # u eo h q f H k
    r = r.reshape(4, 2, 2, 8, 64, 2, 64)        # u eo h j fl H k
    r = r.transpose(0, 1, 2, 3, 5, 4, 6)        # u eo h j H fl k
    return np.ascontiguousarray(r).reshape(NUM_MESHES, F, NUM_KERNEL)


def run(inputs, trace=False):
    from concourse.bass_utils import run_bass_kernel_spmd
    nc = _get_nc()
    centers = np.asarray(inputs["centers"], dtype=np.float32)
    corners = np.asarray(inputs["neighbor_corners"], dtype=np.float32)
    alpha = np.asarray(inputs["alpha"], dtype=np.float32)
    beta = np.asarray(inputs["beta"], dtype=np.float32)
    gamma = np.asarray(inputs["gamma"], dtype=np.float32)
    W = np.asarray(inputs["W"], dtype=np.float32)

    in_maps = [
        _prep_core_inputs(centers, corners, alpha, beta, gamma, W, c)
        for c in range(N_CORES)
    ]
    res = run_bass_kernel_spmd(
        nc, in_maps, core_ids=list(range(N_CORES)), trace=trace)
    out = np.empty((NUM_MESHES, NUM_FACES, NUM_KERNEL), dtype=np.float32)
    for c in range(N_CORES):
        out[:, c * F:(c + 1) * F, :] = _unshuffle_core_out(res.results[c]["out"])
    return out, res


def kernel(**inputs) -> np.ndarray:
    out, _ = run(inputs, trace=False)
    return out



# revision 18
# speedup vs baseline: 3.6994x; 1.2572x over previous
"""Trainium2 Bass kernel for nn_ConvSurface: barycentric surface sampling +
3->64 linear map + ReLU + max over 24 samples.

Sharding: face dimension across 8 cores (alpha/beta/gamma shard too).
Per core: F=2048 faces x M=8 meshes (fm = m*2048 + f, mesh-major).

Device pipeline per core (bf16 compute, f32 PSUM):
  1. DMA in: corn [128,3456] f32 (layout [i,d,f,n] per partition),
     cent [128,384] f32 ([f,d]), coefa/b/g [128,3072] bf16 ([f,s]),
     wblk [6,128] bf16 (block-diag W^T x2)
  2. DVE: cd = corn - cent; gamma-elim: cd0 -= cd2, cd1 -= cd2
  3. DVE per (H,d): dirs = coef0*cd0 + coef1*cd1 + cd2 (4 ops, 2x bf16).
     dirs(H=1) chains are interleaved into quarter (0,0)'s drain loop so
     the DVE never stalls the pipeline mid-kernel.
  4. SBUF->SBUF DMA (gpsimd SWDGE queue, overlaps everything): repack
     dirs into PE rhs layout [rows 32k+3eo+d, fm_local*24]; the (0,0)
     repack is issued per-d so matmuls start ~10us earlier.
  5. PE: fea = dirs . W via 4x row-tiled (32x128) bf16 matmuls, N=384
  6. Drain per pair (4 PSUM quads = 512 outputs), route-mixed:
     'B': 4x ACT relu -> fsb bf16 + 4-op DVE max tree (2x pairwise +
          final reduce-3); 'R': DVE tensor_reduce direct from PSUM.
  7. DMA out bf16 [128=(eo,k), 8192=(rg,g,floc)]; host un-shuffles.
"""

import json
import sys
import types

import numpy as np

sys.path.insert(0, "/opt/trn_rl_repo")

NUM_MESHES = 8
NUM_FACES = 16384
NUM_KERNEL = 64
N_CORES = 8

F = NUM_FACES // N_CORES          # 2048 faces per core
FM = NUM_MESHES * F               # 16384 face-mesh pairs per core
FL = FM // 128                    # 128 fm-items per partition
S = 24

N_MM = 384                        # 16 faces x 24 samples per matmul
FACES_PER_MM = 16

# Per-PAIR drain routes (a pair = 4 PSUM quads = 512 outputs):
# 'R' = DVE tensor_reduce direct from PSUM -> osb (no ACT),
# 'B' = 4x ACT relu -> fsb [g(256), s(24)] + DVE max tree.
# 32 entries (8 per quarter); tuned so DVE/ACT busy times balance.
ROUTE_PATTERN = ['B', 'B', 'R', 'B', 'B', 'B', 'B', 'B',
                 'B', 'B', 'B', 'B', 'B', 'B', 'B', 'B',
                 'B', 'B', 'R', 'B', 'B', 'B', 'B', 'B',
                 'B', 'B', 'B', 'B', 'B', 'B', 'B', 'B']


# --------------------------------------------------------------------------
# Harness patches (wait-split for walrus 1-wait limit; NTFF profiling shim)
# --------------------------------------------------------------------------

def _split_waits(bir: dict) -> dict:
    """walrus codegen accepts at most 1 sync wait per instruction (2 for
    EventSemaphore); Tile sometimes emits more. Move the excess onto NoOp
    carriers inserted just before the instruction on the same engine."""
    n = [0]
    for fn in bir.get("functions", []):
        for bb in fn.get("blocks", []):
            out = []
            for inst in bb.get("instructions", []):
                si = inst.get("sync_info") or {}
                waits = si.get("on_wait") or []
                cap = 2 if inst.get("opcode") == "EventSemaphore" else 1
                if len(waits) > cap:
                    for w in waits[cap:]:
                        n[0] += 1
                        out.append({
                            "name": f"wsplit-{n[0]}",
                            "opcode": "NoOp",
                            "engine": inst.get("engine"),
                            "ins": [], "outs": [],
                            "debug": inst.get("debug"),
                            "sync_info": {"on_update": [], "on_wait": [w]},
                        })
                    si["on_wait"] = waits[:cap]
                    inst["sync_info"] = si
                out.append(inst)
            bb["instructions"] = out
    return bir


def _install_patches():
    import concourse.bass_utils as bu
    import concourse.bass2jax as b2j
    if not getattr(bu, "_wsplit_installed", False):
        orig = bu.compile_bir_kernel

        def wrapper(bir_str, *a, **kw):
            if isinstance(bir_str, (bytes, bytearray)):
                bir_str = json.dumps(_split_waits(json.loads(bir_str))).encode()
            elif isinstance(bir_str, str):
                bir_str = json.dumps(_split_waits(json.loads(bir_str)))
            return orig(bir_str, *a, **kw)

        bu.compile_bir_kernel = wrapper
        b2j.compile_bir_kernel = wrapper
        bu._wsplit_installed = True

    if "antenv.axon_hooks" not in sys.modules:
        mod = types.ModuleType("antenv.axon_hooks")
        _hook = [None]
        mod.set_axon_ntff_profile_hook = lambda h: _hook.__setitem__(0, h)
        mod.get_axon_ntff_profile_hook = lambda: _hook[0]
        sys.modules["antenv.axon_hooks"] = mod
        try:
            import antenv
            antenv.axon_hooks = mod
            from trn_agent_boot.trn_boot import _ntff_profile_via_ctypes
            mod.set_axon_ntff_profile_hook(
                _ntff_profile_via_ctypes("/opt/axon/libaxon_pjrt.so"))
        except Exception:
            pass


# --------------------------------------------------------------------------
# Device kernel
# --------------------------------------------------------------------------

def _merge_ap(ap_obj):
    """Merge adjacent free dims (outer.step == inner.step*inner.count), drop
    count-1 dims -> fit the 3-free-dim ISA mem-pattern limit."""
    import concourse.bass as bass
    pairs = [list(p) for p in ap_obj.ap]
    part, rest = pairs[0], pairs[1:]
    merged = []
    for s, c in rest:
        if c == 1:
            continue
        if merged and merged[-1][0] == s * c:
            merged[-1] = [s, merged[-1][1] * c]
        else:
            merged.append([s, c])
    if not merged:
        merged = [[1, 1]]
    return bass.AP(ap_obj.tensor, ap_obj.offset, [part] + merged)


def _build_nc():
    import concourse.bass as bass
    import concourse.tile as tile
    from concourse import mybir

    f32 = mybir.dt.float32
    bf16 = mybir.dt.bfloat16
    nc = bass.Bass()

    corn_d = nc.declare_dram_parameter("corn", [128, FL * 27], bf16, isOutput=False)
    cent_d = nc.declare_dram_parameter("cent", [128, FL * 3], bf16, isOutput=False)
    coef_d = [nc.declare_dram_parameter(f"coef{i}", [128, FL * S], bf16,
                                        isOutput=False) for i in range(3)]
    wblk_d = nc.declare_dram_parameter("wblk", [6, 128], bf16, isOutput=False)
    out_d = nc.declare_dram_parameter("out", [128, FM // 2], bf16, isOutput=True)

    AX = mybir.AluOpType
    FH = FL // 2                   # f-local per H-half (64)
    RHS_H = 8 * FH * S             # rhs free per (H, h) tile = 12288

    with tile.TileContext(nc) as tc:
        with (
            tc.tile_pool(name="inputs", bufs=1) as inp_pool,
            tc.tile_pool(name="w", bufs=1) as w_pool,
            tc.tile_pool(name="dirs", bufs=2) as dirs_pool,
            tc.tile_pool(name="tmp", bufs=1) as tmp_pool,
            tc.tile_pool(name="rhs", bufs=3) as rhs_pool,
            tc.tile_pool(name="fsb", bufs=3) as fsb_pool,
            tc.tile_pool(name="dtree", bufs=2) as dtree_pool,
            tc.tile_pool(name="osb", bufs=2) as osb_pool,
            tc.tile_pool(name="psum", bufs=2, space="PSUM") as psum_pool,
        ):
            # ---- loads -------------------------------------------------
            corn = inp_pool.tile([128, FL * 27], bf16)    # [i, d, f, n]
            nc.sync.dma_start(corn[:], corn_d[:])
            cent = inp_pool.tile([128, FL * 3], bf16)     # [f, d]
            nc.sync.dma_start(cent[:], cent_d[:])
            coef = []
            for i in range(3):
                t = inp_pool.tile([128, FL * S], bf16, tag=f"coef{i}", name="cf")
                nc.sync.dma_start(t[:], coef_d[i][:])
                coef.append(t)
            wt = w_pool.tile([128, 128], bf16)
            for rg in range(4):
                nc.sync.dma_start(wt[32 * rg:32 * rg + 6, :], wblk_d[:, :])

            # ---- cd = corn - cent (in place, bf16, layout [i, d, f, n]) -
            cr5 = corn[:].rearrange("p (i d f n) -> p i d f n", i=3, d=3, f=FL, n=3)
            ce3 = cent[:].rearrange("p (f d) -> p f d", f=FL, d=3)
            for d in range(3):
                ce = ce3[:, :, d].unsqueeze(1).unsqueeze(3)
                ce = ce.broadcast_to((128, 3, FL, 3))
                nc.vector.tensor_tensor(
                    _merge_ap(cr5[:, :, d, :, :]), _merge_ap(cr5[:, :, d, :, :]),
                    _merge_ap(ce), op=AX.subtract)
            # gamma-elimination (alpha+beta+gamma = 1):
            # dirs = cd2 + a*(cd0-cd2) + b*(cd1-cd2); fold diffs in place.
            for i in range(2):
                nc.vector.tensor_tensor(
                    _merge_ap(cr5[:, i, :, :, :]), _merge_ap(cr5[:, i, :, :, :]),
                    _merge_ap(cr5[:, 2, :, :, :]), op=AX.subtract)

            dirs_of = {}

            def make_dirs_tiles(H):
                dirs_of[H] = [
                    dirs_pool.tile([128, FH * S], bf16, tag=f"dirs{d}",
                                   name="dirs")
                    for d in range(3)
                ]

            def emit_dirs_chain(H, d):
                fsl = slice(H * FH, (H + 1) * FH)
                csl = slice(H * FH * S, (H + 1) * FH * S)

                def cd_ap(i):
                    a = cr5[:, i, d, fsl, :]           # p f n
                    a = a.unsqueeze(2).broadcast_to((128, FH, 8, 3))
                    return _merge_ap(a)

                t1 = tmp_pool.tile([128, FH * S], bf16, tag="t1", name="t1")
                nc.vector.tensor_mul(t1[:], coef[0][:, csl], cd_ap(0))
                t2 = tmp_pool.tile([128, FH * S], bf16, tag="t2", name="t2")
                nc.vector.tensor_mul(t2[:], coef[1][:, csl], cd_ap(1))
                nc.vector.tensor_add(t1[:], t1[:], t2[:])
                nc.vector.tensor_add(dirs_of[H][d][:], t1[:], cd_ap(2))

            def emit_repack(H, h, rhs, d_only=None):
                for d in (range(3) if d_only is None else [d_only]):
                    for eo in range(2):
                        for k in range(4):
                            src = dirs_of[H][d][32 * k + 16 * eo + 8 * h:
                                                32 * k + 16 * eo + 8 * h + 8, :]
                            dst = rhs[32 * k + 3 * eo + d:
                                      32 * k + 3 * eo + d + 1, :]
                            dst = dst.rearrange("p (j c) -> p j c",
                                                j=8, c=FH * S)
                            nc.gpsimd.dma_start(dst, src)

            # ---- prologue: dirs(H=0) with per-d repack of quarter (0,0) -
            make_dirs_tiles(0)
            rhs0 = rhs_pool.tile([128, RHS_H], bf16, name="rhs")
            for d in range(3):
                emit_dirs_chain(0, d)
                emit_repack(0, 0, rhs0, d_only=d)

            pending = []               # dirs(H=1) chains, injected later
            pair_global = 0
            for qi, (H, h) in enumerate([(0, 0), (0, 1), (1, 0), (1, 1)]):
                if qi == 0:
                    rhs = rhs0
                    make_dirs_tiles(1)
                    pending = [lambda d=d: emit_dirs_chain(1, d)
                               for d in range(3)]
                else:
                    rhs = rhs_pool.tile([128, RHS_H], bf16, name="rhs")
                    emit_repack(H, h, rhs)

                osb = osb_pool.tile([128, 2048], bf16)
                for pr in range(8):     # pairs: 4 quads x 4 MMs each
                    route = ROUTE_PATTERN[pair_global]
                    pair_global += 1
                    fsb = (fsb_pool.tile([128, 6144], bf16, name="fsb")
                           if route == "B" else None)
                    for j in range(4):
                        q = pr * 4 + j
                        ps = psum_pool.tile([128, 2048], f32)
                        for u in range(4):   # u = row group (rotates!)
                            nc.tensor.matmul(
                                ps[:, u * 512:u * 512 + N_MM],
                                wt[32 * u:32 * u + 6, :],
                                rhs[32 * u:32 * u + 6,
                                    q * N_MM:(q + 1) * N_MM],
                                start=True, stop=True,
                                tile_position=(32 * u, 0))
                        if route == "R":
                            pa = bass.AP(
                                ps[:].tensor, ps[:].offset,
                                [list(ps[:].ap[0]),
                                 [512, 4], [S, FACES_PER_MM], [1, S]])
                            osl = osb[:, q * 64:(q + 1) * 64]
                            nc.vector.tensor_reduce(
                                osl.rearrange("p (u f) -> p u f", u=4),
                                pa, axis=mybir.AxisListType.X, op=AX.max)
                        else:
                            nc.scalar.activation(
                                fsb[:, j * 1536:(j + 1) * 1536].rearrange(
                                    "p (u c) -> p u c", u=4),
                                bass.AP(ps[:].tensor, ps[:].offset,
                                        [list(ps[:].ap[0]),
                                         [512, 4], [1, N_MM]]),
                                mybir.ActivationFunctionType.Relu)
                    if route == "B":
                        # DVE max tree over pair: [g=256, s=24]
                        osl = osb[:, pr * 256:(pr + 1) * 256]
                        f3 = fsb[:].rearrange("p (g s) -> p g s", g=256, s=S)
                        tr1 = dtree_pool.tile([128, 3072], bf16, tag="tr1")
                        nc.vector.tensor_tensor(
                            tr1[:], _merge_ap(f3[:, :, 0:12]),
                            _merge_ap(f3[:, :, 12:24]), op=AX.max)
                        t13 = tr1[:].rearrange("p (g s) -> p g s", g=256, s=12)
                        tr2 = dtree_pool.tile([128, 1536], bf16, tag="tr2")
                        nc.vector.tensor_tensor(
                            tr2[:], _merge_ap(t13[:, :, 0:6]),
                            _merge_ap(t13[:, :, 6:12]), op=AX.max)
                        t23 = tr2[:].rearrange("p (g s) -> p g s", g=256, s=6)
                        tr3 = dtree_pool.tile([128, 768], bf16, tag="tr3")
                        nc.vector.tensor_tensor(
                            tr3[:], _merge_ap(t23[:, :, 0:3]),
                            _merge_ap(t23[:, :, 3:6]), op=AX.max)
                        nc.vector.tensor_reduce(
                            osl,
                            tr3[:].rearrange("p (g s) -> p g s", g=256, s=3),
                            axis=mybir.AxisListType.X, op=AX.max)
                    # inject a dirs(H=1) chain every other pair of (0,0)
                    if pending and pr % 2 == 1:
                        pending.pop(0)()
                # relu for 'R' pairs (idempotent on B outputs)
                nc.vector.tensor_scalar_max(osb[:], osb[:], 0.0)
                nc.sync.dma_start(
                    out_d[:, (h * 2 + H) * 2048:(h * 2 + H + 1) * 2048],
                    osb[:])
    return nc


_CACHE = {}


def _get_nc():
    if "nc" not in _CACHE:
        _install_patches()
        _CACHE["nc"] = _build_nc()
    return _CACHE["nc"]


# --------------------------------------------------------------------------
# Host wrapper
# --------------------------------------------------------------------------

def _prep_core_inputs(centers, neighbor_corners, alpha, beta, gamma, W, c):
    import ml_dtypes
    bf = ml_dtypes.bfloat16
    fsl = slice(c * F, (c + 1) * F)
    cent = np.ascontiguousarray(
        centers[:, fsl].reshape(128, FL, 3),
        dtype=np.float32).reshape(128, FL * 3).astype(bf)
    # corn per-partition rows [f, n, i, d] -> [i, d, f, n]
    cr = neighbor_corners[:, fsl].reshape(128, FL, 3, 3, 3)
    corn = np.ascontiguousarray(cr.transpose(0, 3, 4, 1, 2), dtype=np.float32)
    corn = corn.reshape(128, FL * 27).astype(bf)
    cf = []
    for arr in (alpha, beta, gamma):
        a = np.tile(arr[fsl].reshape(1, F, S), (NUM_MESHES, 1, 1))
        cf.append(np.ascontiguousarray(
            a.reshape(128, FL * S), dtype=np.float32).astype(bf))
    wblk = np.zeros((6, 128), dtype=np.float32)
    wblk[0:3, 0:64] = W.T
    wblk[3:6, 64:128] = W.T
    return {"corn": corn, "cent": cent,
            "coef0": cf[0], "coef1": cf[1], "coef2": cf[2],
            "wblk": wblk.astype(bf)}


def _unshuffle_core_out(raw):
    # raw [128=(eo,k), 8192]; col = (h*2+H)*2048 + q*64 + u*16 + f
    # stream e = 2u+eo; fm_local = h*1024 + j*128 + H*64 + fl, (q*16+f)=j*64+fl
    r = np.asarray(raw, dtype=np.float32).reshape(2, 64, 2, 2, 32, 4, 16)
    r = r.transpose(5, 0, 2, 4, 6, 3, 1)        # u eo h q f H k
    r = r.reshape(4, 2, 2, 8, 64, 2, 64)        # u eo h j fl H k
    r = r.transpose(0, 1, 2, 3, 5, 4, 6)        # u eo h j H fl k
    return np.ascontiguousarray(r).reshape(NUM_MESHES, F, NUM_KERNEL)


def run(inputs, trace=False):
    from concourse.bass_utils import run_bass_kernel_spmd
    nc = _get_nc()
    centers = np.asarray(inputs["centers"], dtype=np.float32)
    corners = np.asarray(inputs["neighbor_corners"], dtype=np.float32)
    alpha = np.asarray(inputs["alpha"], dtype=np.float32)
    beta = np.asarray(inputs["beta"], dtype=np.float32)
    gamma = np.asarray(inputs["gamma"], dtype=np.float32)
    W = np.asarray(inputs["W"], dtype=np.float32)

    in_maps = [
        _prep_core_inputs(centers, corners, alpha, beta, gamma, W, c)
        for c in range(N_CORES)
    ]
    res = run_bass_kernel_spmd(
        nc, in_maps, core_ids=list(range(N_CORES)), trace=trace)
    out = np.empty((NUM_MESHES, NUM_FACES, NUM_KERNEL), dtype=np.float32)
    for c in range(N_CORES):
        out[:, c * F:(c + 1) * F, :] = _unshuffle_core_out(res.results[c]["out"])
    return out, res


def kernel(**inputs) -> np.ndarray:
    out, _ = run(inputs, trace=False)
    return out


# revision 20
# speedup vs baseline: 3.7837x; 1.0228x over previous
"""Trainium2 Bass kernel for nn_ConvSurface: barycentric surface sampling +
3->64 linear map + ReLU + max over 24 samples.

Sharding: face dimension across 8 cores (alpha/beta/gamma shard too).
Per core: F=2048 faces x M=8 meshes (fm = m*2048 + f, mesh-major).

Device pipeline per core (bf16 compute, f32 PSUM):
  1. DMA in: corn [128,3456] bf16 ([i,d,f,n] per partition), cent
     [128,384] ([f,d]), coef0/1 [128,3072] ([f,s]) on separate queues,
     wblk [6,128] (block-diag W^T x2).
  2. DVE: cd = corn - cent; gamma-elim (cd0 -= cd2, cd1 -= cd2);
     dirs = coef0*cd0 + coef1*cd1 + cd2 per (H,d);
     then in-place s-halving diff: dirs[s<12] -= dirs[s>=12].
  3. SBUF->SBUF repack DMA (gpsimd SWDGE, fully overlapped): dirs ->
     PE rhs rows (32k+3eo+d); issued per-d in the prologue.
  4. Level-1 max via PE+ACT (max(lo,hi) = relu(lo-hi)+hi):
     PE pass 1: (dirs_lo - dirs_hi).W -> PSUM-A (2-bank tiles);
     ACT: Relu PSUM-A -> PSUM-B;
     PE pass 2: dirs_hi.W accumulated onto PSUM-B (start=False).
     PSUM-B then holds the 12 level-1 maxes per (face, k).
  5. Drain PSUM-B per ss-unit, route-mixed for ACT/DVE balance:
     default: DVE tensor_reduce over s=12 -> osb;
     assist:  ACT relu -> SBUF bf16 + 3-op DVE tree.
  6. Final relu on osb (4x tensor_scalar), DMA out bf16 [128, 8192].
"""

import json
import sys
import types

import numpy as np

sys.path.insert(0, "/opt/trn_rl_repo")

NUM_MESHES = 8
NUM_FACES = 16384
NUM_KERNEL = 64
N_CORES = 8

F = NUM_FACES // N_CORES          # 2048 faces per core
FM = NUM_MESHES * F               # 16384 face-mesh pairs per core
FL = FM // 128                    # 128 fm-items per partition
S = 24

N_MM = 384

# Drain routing per ss-unit (16 per quarter): True = ACT-assisted tree,
# False = DVE tensor_reduce direct. Tuned for ACT/DVE busy balance.
ASSIST_PATTERN = [False, True, False, False,
                  True, False, False, True,
                  False, False, True, False,
                  False, True, False, False]


# --------------------------------------------------------------------------
# Harness patches (wait-split for walrus 1-wait limit; NTFF profiling shim)
# --------------------------------------------------------------------------

def _split_waits(bir: dict) -> dict:
    """walrus codegen accepts at most 1 sync wait per instruction (2 for
    EventSemaphore); Tile sometimes emits more. Move the excess onto NoOp
    carriers inserted just before the instruction on the same engine."""
    n = [0]
    for fn in bir.get("functions", []):
        for bb in fn.get("blocks", []):
            out = []
            for inst in bb.get("instructions", []):
                si = inst.get("sync_info") or {}
                waits = si.get("on_wait") or []
                cap = 2 if inst.get("opcode") == "EventSemaphore" else 1
                if len(waits) > cap:
                    for w in waits[cap:]:
                        n[0] += 1
                        out.append({
                            "name": f"wsplit-{n[0]}",
                            "opcode": "NoOp",
                            "engine": inst.get("engine"),
                            "ins": [], "outs": [],
                            "debug": inst.get("debug"),
                            "sync_info": {"on_update": [], "on_wait": [w]},
                        })
                    si["on_wait"] = waits[:cap]
                    inst["sync_info"] = si
                out.append(inst)
            bb["instructions"] = out
    return bir


def _install_patches():
    import concourse.bass_utils as bu
    import concourse.bass2jax as b2j
    if not getattr(bu, "_wsplit_installed", False):
        orig = bu.compile_bir_kernel

        def wrapper(bir_str, *a, **kw):
            if isinstance(bir_str, (bytes, bytearray)):
                bir_str = json.dumps(_split_waits(json.loads(bir_str))).encode()
            elif isinstance(bir_str, str):
                bir_str = json.dumps(_split_waits(json.loads(bir_str)))
            return orig(bir_str, *a, **kw)

        bu.compile_bir_kernel = wrapper
        b2j.compile_bir_kernel = wrapper
        bu._wsplit_installed = True

    if "antenv.axon_hooks" not in sys.modules:
        mod = types.ModuleType("antenv.axon_hooks")
        _hook = [None]
        mod.set_axon_ntff_profile_hook = lambda h: _hook.__setitem__(0, h)
        mod.get_axon_ntff_profile_hook = lambda: _hook[0]
        sys.modules["antenv.axon_hooks"] = mod
        try:
            import antenv
            antenv.axon_hooks = mod
            from trn_agent_boot.trn_boot import _ntff_profile_via_ctypes
            mod.set_axon_ntff_profile_hook(
                _ntff_profile_via_ctypes("/opt/axon/libaxon_pjrt.so"))
        except Exception:
            pass


# --------------------------------------------------------------------------
# Device kernel
# --------------------------------------------------------------------------

def _merge_ap(ap_obj):
    """Merge adjacent free dims (outer.step == inner.step*inner.count), drop
    count-1 dims -> fit the 3-free-dim ISA mem-pattern limit."""
    import concourse.bass as bass
    pairs = [list(p) for p in ap_obj.ap]
    part, rest = pairs[0], pairs[1:]
    merged = []
    for s, c in rest:
        if c == 1:
            continue
        if merged and merged[-1][0] == s * c:
            merged[-1] = [s, merged[-1][1] * c]
        else:
            merged.append([s, c])
    if not merged:
        merged = [[1, 1]]
    return bass.AP(ap_obj.tensor, ap_obj.offset, [part] + merged)


def _build_nc():
    import concourse.bass as bass
    import concourse.tile as tile
    from concourse import mybir

    f32 = mybir.dt.float32
    bf16 = mybir.dt.bfloat16
    nc = bass.Bass()

    corn_d = nc.declare_dram_parameter("corn", [128, FL * 27], bf16, isOutput=False)
    cent_d = nc.declare_dram_parameter("cent", [128, FL * 3], bf16, isOutput=False)
    coef_d = [nc.declare_dram_parameter(f"coef{i}", [128, FL * S], bf16,
                                        isOutput=False) for i in range(2)]
    wblk_d = nc.declare_dram_parameter("wblk", [6, 128], bf16, isOutput=False)
    out_d = nc.declare_dram_parameter("out", [128, FM // 2], bf16, isOutput=True)

    AX = mybir.AluOpType
    FH = FL // 2                   # f-local per H-half (64)
    RHS_H = 8 * FH * S             # rhs free per (H, h) tile = 12288

    with tile.TileContext(nc) as tc:
        with (
            tc.tile_pool(name="inputs", bufs=1) as inp_pool,
            tc.tile_pool(name="w", bufs=1) as w_pool,
            tc.tile_pool(name="dirs", bufs=2) as dirs_pool,
            tc.tile_pool(name="tmp", bufs=1) as tmp_pool,
            tc.tile_pool(name="rhs", bufs=3) as rhs_pool,
            tc.tile_pool(name="fsb", bufs=3) as fsb_pool,
            tc.tile_pool(name="dtree", bufs=2) as dtree_pool,
            tc.tile_pool(name="osb", bufs=2) as osb_pool,
            tc.tile_pool(name="psumA", bufs=2, space="PSUM") as psA_pool,
            tc.tile_pool(name="psumB", bufs=2, space="PSUM") as psB_pool,
        ):
            # ---- loads (corn/cent first; coefs on other queues) ---------
            corn = inp_pool.tile([128, FL * 27], bf16)    # [i, d, f, n]
            nc.sync.dma_start(corn[:], corn_d[:])
            cent = inp_pool.tile([128, FL * 3], bf16)     # [f, d]
            nc.sync.dma_start(cent[:], cent_d[:])
            coef = []
            for i in range(2):
                t = inp_pool.tile([128, FL * S], bf16, tag=f"coef{i}", name="cf")
                nc.scalar.dma_start(t[:], coef_d[i][:])
                coef.append(t)
            wt = w_pool.tile([128, 128], bf16)
            for rg in range(4):
                nc.sync.dma_start(wt[32 * rg:32 * rg + 6, :], wblk_d[:, :])

            # ---- cd = corn - cent (in place, bf16, layout [i, d, f, n]) -
            cr5 = corn[:].rearrange("p (i d f n) -> p i d f n", i=3, d=3, f=FL, n=3)
            ce3 = cent[:].rearrange("p (f d) -> p f d", f=FL, d=3)
            for d in range(3):
                ce = ce3[:, :, d].unsqueeze(1).unsqueeze(3)
                ce = ce.broadcast_to((128, 3, FL, 3))
                nc.vector.tensor_tensor(
                    _merge_ap(cr5[:, :, d, :, :]), _merge_ap(cr5[:, :, d, :, :]),
                    _merge_ap(ce), op=AX.subtract)
            # gamma-elimination (alpha+beta+gamma = 1):
            # dirs = cd2 + a*(cd0-cd2) + b*(cd1-cd2); fold diffs in place.
            for i in range(2):
                nc.vector.tensor_tensor(
                    _merge_ap(cr5[:, i, :, :, :]), _merge_ap(cr5[:, i, :, :, :]),
                    _merge_ap(cr5[:, 2, :, :, :]), op=AX.subtract)

            dirs_of = {}

            def make_dirs_tiles(H):
                dirs_of[H] = [
                    dirs_pool.tile([128, FH * S], bf16, tag=f"dirs{d}",
                                   name="dirs")
                    for d in range(3)
                ]

            def emit_dirs_chain(H, d):
                fsl = slice(H * FH, (H + 1) * FH)
                csl = slice(H * FH * S, (H + 1) * FH * S)

                def cd_ap(i):
                    a = cr5[:, i, d, fsl, :]           # p f n
                    a = a.unsqueeze(2).broadcast_to((128, FH, 8, 3))
                    return _merge_ap(a)

                t1 = tmp_pool.tile([128, FH * S], bf16, tag="t1", name="t1")
                nc.vector.tensor_mul(t1[:], coef[0][:, csl], cd_ap(0))
                t2 = tmp_pool.tile([128, FH * S], bf16, tag="t2", name="t2")
                nc.vector.tensor_mul(t2[:], coef[1][:, csl], cd_ap(1))
                nc.vector.tensor_add(t1[:], t1[:], t2[:])
                nc.vector.tensor_add(dirs_of[H][d][:], t1[:], cd_ap(2))
                # s-halving diff for the PE+ACT level-1 max:
                # dirs[:, f, 0:12] -= dirs[:, f, 12:24]
                dv = dirs_of[H][d][:].rearrange("p (f s) -> p f s", f=FH, s=S)
                nc.vector.tensor_tensor(
                    _merge_ap(dv[:, :, 0:12]), _merge_ap(dv[:, :, 0:12]),
                    _merge_ap(dv[:, :, 12:24]), op=AX.subtract)

            def emit_repack(H, h, rhs, d_only=None):
                for d in (range(3) if d_only is None else [d_only]):
                    for eo in range(2):
                        for k in range(4):
                            src = dirs_of[H][d][32 * k + 16 * eo + 8 * h:
                                                32 * k + 16 * eo + 8 * h + 8, :]
                            dst = rhs[32 * k + 3 * eo + d:
                                      32 * k + 3 * eo + d + 1, :]
                            dst = dst.rearrange("p (j c) -> p j c",
                                                j=8, c=FH * S)
                            nc.gpsimd.dma_start(dst, src)

            # ---- prologue: dirs(H=0) with per-d repack of quarter (0,0) -
            make_dirs_tiles(0)
            rhs0 = rhs_pool.tile([128, RHS_H], bf16, name="rhs")
            for d in range(3):
                emit_dirs_chain(0, d)
                emit_repack(0, 0, rhs0, d_only=d)

            pending = []               # dirs(H=1) chains, injected later
            for qi, (H, h) in enumerate([(0, 0), (0, 1), (1, 0), (1, 1)]):
                if qi == 0:
                    rhs = rhs0
                    make_dirs_tiles(1)
                    pending = [lambda d=d: emit_dirs_chain(1, d)
                               for d in range(3)]
                else:
                    rhs = rhs_pool.tile([128, RHS_H], bf16, name="rhs")
                    emit_repack(H, h, rhs)

                rv = rhs[:].rearrange("p (j f s) -> p j f s", j=8, f=FH, s=S)
                osb = osb_pool.tile([128, 2048], bf16)
                for ss in range(16):            # ss = j*2 + fb
                    j, fb = ss // 2, ss % 2
                    assist = ASSIST_PATTERN[ss]
                    fsb = (fsb_pool.tile([128, 1536], bf16, name="fsb")
                           if assist else None)
                    for t in range(2):          # u-pair
                        psA = psA_pool.tile([128, 1024], f32)
                        for uu in range(2):
                            u = 2 * t + uu
                            lo = rv[32 * u:32 * u + 6, j,
                                    fb * 32:fb * 32 + 32, 0:12]
                            nc.tensor.matmul(
                                psA[:, uu * 512:uu * 512 + N_MM],
                                wt[32 * u:32 * u + 6, :], lo,
                                start=True, stop=True,
                                tile_position=(32 * u, 0))
                        psB = psB_pool.tile([128, 1024], f32)
                        av = psA[:].rearrange("p (uu c) -> p uu c", uu=2)
                        bv = psB[:].rearrange("p (uu c) -> p uu c", uu=2)
                        nc.scalar.activation(
                            bv[:, :, 0:N_MM], av[:, :, 0:N_MM],
                            mybir.ActivationFunctionType.Relu)
                        for uu in range(2):
                            u = 2 * t + uu
                            hi = rv[32 * u:32 * u + 6, j,
                                    fb * 32:fb * 32 + 32, 12:24]
                            nc.tensor.matmul(
                                psB[:, uu * 512:uu * 512 + N_MM],
                                wt[32 * u:32 * u + 6, :], hi,
                                start=False, stop=True,
                                skip_group_check=True,
                                tile_position=(32 * u, 0))
                        # PSUM-B now holds 12 level-1 maxes per group
                        b4 = bv[:, :, 0:N_MM].rearrange(
                            "p uu (f s) -> p uu f s", f=32, s=12)
                        if assist:
                            nc.scalar.activation(
                                fsb[:, t * 768:(t + 1) * 768].rearrange(
                                    "p (uu f s) -> p uu f s", uu=2, f=32),
                                b4, mybir.ActivationFunctionType.Relu)
                        else:
                            osl = osb[:, ss * 128 + t * 64:
                                      ss * 128 + t * 64 + 64]
                            nc.vector.tensor_reduce(
                                osl.rearrange("p (uu f) -> p uu f", uu=2),
                                b4, axis=mybir.AxisListType.X, op=AX.max)
                    if assist:
                        # DVE tree from s=12: [g=256, 12] -> 6 -> 3 -> 1
                        f3 = fsb[:].rearrange("p (g s) -> p g s", g=128, s=12)
                        tr1 = dtree_pool.tile([128, 768], bf16, tag="tr1")
                        nc.vector.tensor_tensor(
                            tr1[:], _merge_ap(f3[:, :, 0:6]),
                            _merge_ap(f3[:, :, 6:12]), op=AX.max)
                        t13 = tr1[:].rearrange("p (g s) -> p g s", g=128, s=6)
                        tr2 = dtree_pool.tile([128, 384], bf16, tag="tr2")
                        nc.vector.tensor_tensor(
                            tr2[:], _merge_ap(t13[:, :, 0:3]),
                            _merge_ap(t13[:, :, 3:6]), op=AX.max)
                        nc.vector.tensor_reduce(
                            osb[:, ss * 128:(ss + 1) * 128],
                            tr2[:].rearrange("p (g s) -> p g s", g=128, s=3),
                            axis=mybir.AxisListType.X, op=AX.max)
                    # inject a dirs(H=1) chain every few ss of quarter 0
                    if pending and ss % 4 == 3:
                        pending.pop(0)()
                # final relu (R-route values are raw maxes)
                nc.vector.tensor_scalar_max(osb[:], osb[:], 0.0)
                nc.sync.dma_start(
                    out_d[:, (h * 2 + H) * 2048:(h * 2 + H + 1) * 2048],
                    osb[:])
    return nc


_CACHE = {}


def _get_nc():
    if "nc" not in _CACHE:
        _install_patches()
        _CACHE["nc"] = _build_nc()
    return _CACHE["nc"]


# --------------------------------------------------------------------------
# Host wrapper
# --------------------------------------------------------------------------

def _prep_core_inputs(centers, neighbor_corners, alpha, beta, W, c):
    import ml_dtypes
    bf = ml_dtypes.bfloat16
    fsl = slice(c * F, (c + 1) * F)
    cent = np.ascontiguousarray(
        centers[:, fsl].reshape(128, FL, 3),
        dtype=np.float32).reshape(128, FL * 3).astype(bf)
    # corn per-partition rows [f, n, i, d] -> [i, d, f, n]
    cr = neighbor_corners[:, fsl].reshape(128, FL, 3, 3, 3)
    corn = np.ascontiguousarray(cr.transpose(0, 3, 4, 1, 2), dtype=np.float32)
    corn = corn.reshape(128, FL * 27).astype(bf)
    cf = []
    for arr in (alpha, beta):
        a = np.tile(arr[fsl].reshape(1, F, S), (NUM_MESHES, 1, 1))
        cf.append(np.ascontiguousarray(
            a.reshape(128, FL * S), dtype=np.float32).astype(bf))
    wblk = np.zeros((6, 128), dtype=np.float32)
    wblk[0:3, 0:64] = W.T
    wblk[3:6, 64:128] = W.T
    return {"corn": corn, "cent": cent,
            "coef0": cf[0], "coef1": cf[1],
            "wblk": wblk.astype(bf)}


_IDX_CACHE = {}


def _gather_idx():
    """(m, f_local, k) index arrays for the [128, 8192] core output.

    part p = eo*64 + k.  col c: Q = c//2048 -> (h = Q//2, H = Q%2);
    rem = c%2048: ss = rem//128 (j = ss//2, fb = ss%2); r2 = rem%128:
    t = r2//64, uu = (r2%64)//32, f32 = r2%32; u = 2t+uu.
    dirs row = 32u+16eo+8h+j, fl = H*64 + fb*32 + f32;
    fm flat = row*128 + fl -> m = flat//F, f_local = flat%F.
    """
    if "idx" not in _IDX_CACHE:
        p = np.arange(128)[:, None]
        c = np.arange(8192)[None, :]
        eo, k = p // 64, p % 64
        Q, rem = c // 2048, c % 2048
        h, H = Q // 2, Q % 2
        ss, r2 = rem // 128, rem % 128
        j, fb = ss // 2, ss % 2
        t, uu, f32 = r2 // 64, (r2 % 64) // 32, r2 % 32
        u = 2 * t + uu
        row = 32 * u + 16 * eo + 8 * h + j
        fl = H * 64 + fb * 32 + f32
        flat = row * 128 + fl
        m = flat // F
        f_local = flat % F
        kk = np.broadcast_to(k, (128, 8192))
        _IDX_CACHE["idx"] = (m, f_local, kk)
    return _IDX_CACHE["idx"]


def _unshuffle_core_out(raw):
    m, f_local, kk = _gather_idx()
    out = np.empty((NUM_MESHES, F, NUM_KERNEL), dtype=np.float32)
    out[m, f_local, kk] = np.asarray(raw, dtype=np.float32)
    return out


def run(inputs, trace=False):
    from concourse.bass_utils import run_bass_kernel_spmd
    nc = _get_nc()
    centers = np.asarray(inputs["centers"], dtype=np.float32)
    corners = np.asarray(inputs["neighbor_corners"], dtype=np.float32)
    alpha = np.asarray(inputs["alpha"], dtype=np.float32)
    beta = np.asarray(inputs["beta"], dtype=np.float32)
    W = np.asarray(inputs["W"], dtype=np.float32)

    in_maps = [
        _prep_core_inputs(centers, corners, alpha, beta, W, c)
        for c in range(N_CORES)
    ]
    res = run_bass_kernel_spmd(
        nc, in_maps, core_ids=list(range(N_CORES)), trace=trace)
    out = np.empty((NUM_MESHES, NUM_FACES, NUM_KERNEL), dtype=np.float32)
    for c in range(N_CORES):
        out[:, c * F:(c + 1) * F, :] = _unshuffle_core_out(res.results[c]["out"])
    return out, res


def kernel(**inputs) -> np.ndarray:
    out, _ = run(inputs, trace=False)
    return out
